# revision 7
# baseline (speedup 1.0000x reference)
"""Trainium2 Bass kernel for the CIN-style layer:

    z   = einsum('btf,byf->bfty', x_0, x_k)            # pairwise outer products
    z   = z.reshape(bs, ts0, f, tsk)                   # flat reinterpretation
    out = einsum('btiy,nty->bni', z, conv_w) + conv_b  # strided conv reduction

Shapes: x_0 (32, 64, 256), x_k (32, 64, 256), conv_w (128, 64, 64),
conv_b (128,) -> out (32, 128, 256).

Math: with i = a*64 + m  (a = i//64, m = i%64) and feature f = 4t + a the
reference reduces to a two-step factorization:

    W2[b,n,t,a]      = sum_y x_k[b,y,4t+a] * conv_w[n,t,y]         (contract y)
    out[b,n,a*64+m]  = sum_t x_0[b,m,4t+a] * W2[b,n,t,a] + conv_b  (contract t)

Sharding: pure data parallel over batch, 4 samples per core, conv_w/conv_b
replicated (no collectives).

v24 (bf16): everything bf16 on the wire (tolerance 2e-2, bf16 end-to-end
measures ~5e-3). xk/x0 ship host-side zero-padded (on-chip padding via
GpSimd/DVE measured slower than the extra 0.7us of stream). SDMA engines
drain queues with strict per-engine priority, so splitting the input
across queues buys nothing; instead ONE big-row input stream (12.5KB/row)
goes on sync, ordered [xk | wt | x0 | bias], and the scalar ring is kept
free for bounce traffic. The W2 shuffle bounces through DRAM as a straight
contiguous dump with the permutation on the (slow, 256B-descriptor)
readback side; the readback is split into h-halves running on both rings
in parallel, the two down halves also go on separate rings, and the last
PSUM cast is split so only a 128-col cast gates the second down.

Device mapping (per core, c = 4*b_loc + a in [0,16), t = 2p + q, p = 4k + j):
  step 1: 32 matmuls, one per t-pair p: stationary lhsT = zero-padded xk tile
          [K=128 (q,y), M=32 (q',c)], moving rhs = host-pre-transposed conv_w
          tile [128 (q,y), 128 n]. Four pairs (j=0..3) share a pass k via
          column tiling -> PSUM [32j+16q'+c, n] per pass.
  step 2: 8 matmuls, one per c-pair c2: stationary lhsT = zero-padded x0 tile
          [K=128 (h,t), M=128 (h',m)], moving rhs = shuffled W2 tile
          [128 (h,t), 128 n] -> PSUM [64h+m, n]; bias fused into the
          PSUM->SBUF cast on the vector engine.
"""

import ml_dtypes
import numpy as np

BS, TS, F, NF = 32, 64, 256, 128
NCORES = 8
B = BS // NCORES  # 4 local batches per core

F32 = np.float32
BF16 = ml_dtypes.bfloat16


# ---------------------------------------------------------------------------
# Host-side packing
# ---------------------------------------------------------------------------

def _pack_wt(conv_w: np.ndarray) -> np.ndarray:
    # WT[64q+y, 128p+n] = conv_w[n, 2p+q, y]
    wt = conv_w.transpose(1, 2, 0).reshape(32, 2, 64, NF)  # [p, q, y, n]
    wt = wt.transpose(1, 2, 0, 3)                          # [q, y, p, n]
    return np.ascontiguousarray(wt.reshape(128, 32 * NF)).astype(BF16)


def _pack_xk(xk_shard: np.ndarray) -> np.ndarray:
    # XK[64q+y, 32p+16q'+c] = xk[b, y, 8p+4q+a] iff q'==q else 0  (c = 4b+a)
    xq = xk_shard.reshape(B, TS, 32, 2, 4)       # [b, y, p, q, a]
    src = xq.transpose(3, 1, 2, 0, 4)            # [q, y, p, b, a]
    arr = np.zeros((2, TS, 32, 2, B, 4), dtype=F32)
    arr[0, :, :, 0] = src[0]
    arr[1, :, :, 1] = src[1]
    return arr.reshape(128, 1024).astype(BF16)


def _pack_x0(x0_shard: np.ndarray) -> np.ndarray:
    # X0L[64h+t, 128c2+64h'+m] = x0[b(c), m, 4t+a(c)] iff h'==h  (c = 2c2+h)
    xt = x0_shard.reshape(B, TS, TS, 4).transpose(0, 3, 2, 1)  # [b, a, t, m]
    flat = xt.reshape(16, TS, TS)                              # [c, t, m]
    arr = np.zeros((2, TS, 8, 2, TS), dtype=F32)               # [h, t, c2, h', m]
    for h in (0, 1):
        arr[h, :, :, h, :] = flat[2 * np.arange(8) + h].transpose(1, 0, 2)
    return arr.reshape(128, 1024).astype(BF16)


def _unpack_out(out_pack: np.ndarray, out_full: np.ndarray, r: int) -> None:
    # out_pack[64h+m, 128c2+n] = out[4r+b(c), n, a(c)*64+m], c = 2*c2+h
    o = np.asarray(out_pack).astype(F32).reshape(2, TS, 8, NF)  # [h, m, c2, n]
    for c2 in range(8):
        for h in (0, 1):
            c = 2 * c2 + h
            b, a = divmod(c, 4)
            out_full[4 * r + b, :, a * TS:(a + 1) * TS] = o[h, :, c2, :].T


# ---------------------------------------------------------------------------
# Device program
# ---------------------------------------------------------------------------

_prog_cache = {}


def _emit_body(nc, tc, pool, ps_pool, in_d, in2_d, out_d, w2b_d, version=24):
    import concourse.mybir as mybir

    bf = mybir.dt.bfloat16
    f32 = mybir.dt.float32

    # PE warm-up: back-to-back matmuls on a zeroed bf16 tile while the input
    # DMAs stream in; gets the HAM clock gate to 2.4GHz before step 1 starts.
    warm_s = pool.tile([128, 512], bf, tag="warm")
    nc.vector.memset(warm_s[:], 0.0)
    ps_w = ps_pool.tile([128, 512], f32, tag="warm_ps")
    for _ in range(6):
        nc.tensor.matmul(ps_w[:, :], warm_s[:, 0:128], warm_s[:, :],
                         start=True, stop=True)

    # single big-row input stream on sync, ordered by first use; the scalar
    # ring stays free for bounce traffic
    in_s = pool.tile([128, 6272], bf, tag="in")  # [xk_pad | wt | x0_pad | bias]
    nc.sync.dma_start(in_s[:], in_d.ap())

    xk_pad = in_s[:, 0:1024]
    x0_pad = in_s[:, 5120:6144]
    bias_s = in_s[:, 6144:6272]

    def wt_cols(p):  # rhs tile [128, 128] for pair p
        return in_s[:, 1024 + 128 * p:1024 + 128 * (p + 1)]

    # ---- step 1: W2 = xk . wT, contract y (K = 128 = (q, y)) ----
    w2_s = pool.tile([128, 1024], bf, tag="w2")
    w2r_s = pool.tile([128, 1024], bf, tag="w2r")

    ps1_0 = ps_pool.tile([128, 512], f32, tag="s1_0")
    ps1_1 = ps_pool.tile([128, 512], f32, tag="s1_1")
    ps1s = [ps1_0, ps1_1]
    for u in range(2):
        ps1 = ps1s[u]
        for k in range(4 * u, 4 * u + 4):
            for j in range(4):
                p = 4 * k + j
                nc.tensor.matmul(
                    ps1[32 * j:32 * (j + 1), 128 * (k % 4):128 * (k % 4 + 1)],
                    xk_pad[:, 32 * p:32 * (p + 1)],
                    wt_cols(p),
                    start=True,
                    stop=True,
                    tile_position=(0, 32 * j),
                )
        # split the PSUM cast so only a thin 128-col cast gates the second
        # down DMA (the bulk cast overlaps the last pass's matmuls)
        if u == 0:
            nc.vector.tensor_copy(w2_s[:, 0:512], ps1[:, :])
        else:
            nc.vector.tensor_copy(w2_s[:, 512:896], ps1[:, 0:384])
            nc.vector.tensor_copy(w2_s[:, 896:1024], ps1[:, 384:512])
        # straight contiguous dump of this half (partition (j,q,c2,h),
        # free (k,n)); the permutation happens on the readback below
        (nc.sync if u == 0 else nc.scalar).dma_start(
            w2b_d.ap()[:, 512 * u:512 * (u + 1)],
            w2_s[:, 512 * u:512 * (u + 1)])

    # ---- shuffle readback: partition (j,q,c2,h) -> (h,k,j,q) = (h,t),
    # split into h-halves running on both rings in parallel ----
    src = w2b_d.ap().rearrange("(j q c2 h) (k n) -> h k j q c2 n",
                               j=4, q=2, c2=8, h=2, k=8, n=128)
    for h in range(2):
        dst = w2r_s[64 * h:64 * (h + 1), :].rearrange(
            "p (c2 n) -> p c2 n", c2=8)
        (nc.scalar if h == 0 else nc.sync).dma_start(dst, src[h])

    # ---- step 2: out = x0 . W2, contract t (K = 128 = (h, t)) ----
    out_s = pool.tile([128, 1024], bf, tag="out")
    bias4 = bias_s.unsqueeze(1).broadcast_to([128, 4, 128])
    for u in range(2):
        ps2 = ps_pool.tile([128, 512], f32, tag=f"s2_{u}")
        for c2 in range(4 * u, 4 * u + 4):
            nc.tensor.matmul(
                ps2[:, 128 * (c2 % 4):128 * (c2 % 4 + 1)],
                x0_pad[:, 128 * c2:128 * (c2 + 1)],
                w2r_s[:, 128 * c2:128 * (c2 + 1)],
                start=True,
                stop=True,
            )
        nc.vector.tensor_add(
            out_s[:, 512 * u:512 * (u + 1)].rearrange("p (f n) -> p f n", f=4),
            ps2[:, :].rearrange("p (f n) -> p f n", f=4),
            bias4,
        )
        (nc.sync if u == 0 else nc.scalar).dma_start(
            out_d.ap()[:, 512 * u:512 * (u + 1)],
            out_s[:, 512 * u:512 * (u + 1)])


def _build_program(version=24):
    if version in _prog_cache:
        return _prog_cache[version]

    from contextlib import ExitStack

    import concourse.bacc as bacc
    import concourse.mybir as mybir
    import concourse.tile as tile

    bf = mybir.dt.bfloat16
    nc = bacc.Bacc("TRN2", target_bir_lowering=False, debug=False)

    # in = [xk_pad (1024) | wt (4096) | x0_pad (1024) | bias (128)]
    in_d = nc.dram_tensor("in_pack", [128, 6272], bf, kind="ExternalInput")
    in2_d = None
    out_d = nc.dram_tensor("out_pack", [128, 1024], bf, kind="ExternalOutput")
    # bounce buffer, straight dump of w2_s: [(j q c2 h), (k n)]
    w2b_d = nc.dram_tensor("w2_bounce", [128, 1024], bf)

    with tile.TileContext(nc) as tc, ExitStack() as ctx:
        pool = ctx.enter_context(tc.tile_pool(name="io", bufs=1))
        ps_pool = ctx.enter_context(tc.tile_pool(name="ps", bufs=1, space="PSUM"))
        _emit_body(nc, tc, pool, ps_pool, in_d, in2_d, out_d, w2b_d,
                   version=version)  # in2_d unused in v24

    nc.compile()
    _prog_cache[version] = nc
    return nc


def pack_core_inputs(x_0, x_k, conv_w, conv_b, version=24):
    """Returns (in_maps list of 8 dicts) for run_bass_kernel_spmd."""
    wt = _pack_wt(np.asarray(conv_w, dtype=F32))
    bias = np.ascontiguousarray(
        np.broadcast_to(np.asarray(conv_b, dtype=F32), (128, 128))
    ).astype(BF16)
    x0 = np.asarray(x_0, dtype=F32)
    xk = np.asarray(x_k, dtype=F32)
    in_maps = []
    for r in range(NCORES):
        in_pack = np.concatenate(
            [_pack_xk(xk[B * r:B * (r + 1)]), wt,
             _pack_x0(x0[B * r:B * (r + 1)]), bias], axis=1)
        in_maps.append({
            "in_pack": np.ascontiguousarray(in_pack),
        })
    return in_maps


VERSION = 24  # current best variant


def kernel(x_0, x_k, conv_w, conv_b):
    from concourse.bass_utils import run_bass_kernel_spmd

    nc = _build_program(VERSION)
    in_maps = pack_core_inputs(x_0, x_k, conv_w, conv_b, version=VERSION)
    res = run_bass_kernel_spmd(nc, in_maps, core_ids=list(range(NCORES)))
    out = np.empty((BS, NF, F), dtype=F32)
    for r in range(NCORES):
        _unpack_out(res.results[r]["out_pack"], out, r)
    return out


# ---------------------------------------------------------------------------
# numpy model of the packed device program (for testing the packing logic)
# ---------------------------------------------------------------------------

def _pad_xk(xkd: np.ndarray) -> np.ndarray:
    arr = np.zeros((2, TS, 32, 2, 16), dtype=F32)  # [q, y, p, q', c]
    d = xkd.astype(F32).reshape(2, TS, 32, 16)     # [q, y, p, c]
    for q in (0, 1):
        arr[q, :, :, q] = d[q]
    return arr.reshape(128, 1024)


def _pad_x0(x0d: np.ndarray) -> np.ndarray:
    arr = np.zeros((2, TS, 8, 2, TS), dtype=F32)   # [h, t, c2, h', m]
    d = x0d.astype(F32).reshape(2, TS, 8, TS)      # [h, t, c2, m]
    for h in (0, 1):
        arr[h, :, :, h] = d[h]
    return arr.reshape(128, 1024)


def _numpy_model(x_0, x_k, conv_w, conv_b):
    out = np.empty((BS, NF, F), dtype=F32)
    in_maps = pack_core_inputs(x_0, x_k, conv_w, conv_b)
    for r in range(NCORES):
        m = in_maps[r]
        ip = m["in_pack"].astype(F32)
        xk_s = ip[:, :1024]
        wt = ip[:, 1024:5120]
        x0l = ip[:, 5120:6144]
        bias = ip[:, 6144:6272]
        w2 = np.zeros((128, 1024), dtype=F32)
        for k in range(8):
            ps1 = np.zeros((128, 128), dtype=F32)
            for j in range(4):
                p = 4 * k + j
                ps1[32 * j:32 * (j + 1), :] = (
                    xk_s[:, 32 * p:32 * (p + 1)].T @ wt[:, 128 * p:128 * (p + 1)]
                )
            w2[:, 128 * k:128 * (k + 1)] = ps1
        w2 = w2.astype(BF16).astype(F32)  # PSUM->SBUF bf16 cast
        # bounce: straight dump, permuting readback -> partition (h,k,j,q)
        srcA = w2.reshape(4, 2, 8, 2, 8, 128)          # [j,q,c2,h,k,n]
        w2b = srcA.transpose(3, 4, 0, 1, 2, 5)         # [h,k,j,q,c2,n]
        w2r = w2b.reshape(128, 1024)
        out_pack = np.empty((128, 1024), dtype=F32)
        for c2 in range(8):
            out_pack[:, 128 * c2:128 * (c2 + 1)] = (
                x0l[:, 128 * c2:128 * (c2 + 1)].T @ w2r[:, 128 * c2:128 * (c2 + 1)]
                + bias
            )
        _unpack_out(out_pack.astype(BF16), out, r)
    return out


# revision 8
# speedup vs baseline: 1.0264x; 1.0264x over previous
"""Trainium2 Bass kernel for the CIN-style layer:

    z   = einsum('btf,byf->bfty', x_0, x_k)            # pairwise outer products
    z   = z.reshape(bs, ts0, f, tsk)                   # flat reinterpretation
    out = einsum('btiy,nty->bni', z, conv_w) + conv_b  # strided conv reduction

Shapes: x_0 (32, 64, 256), x_k (32, 64, 256), conv_w (128, 64, 64),
conv_b (128,) -> out (32, 128, 256).

Math: with i = a*64 + m  (a = i//64, m = i%64) and feature f = 4t + a the
reference reduces to a two-step factorization:

    W2[b,n,t,a]      = sum_y x_k[b,y,4t+a] * conv_w[n,t,y]         (contract y)
    out[b,n,a*64+m]  = sum_t x_0[b,m,4t+a] * W2[b,n,t,a] + conv_b  (contract t)

Sharding: pure data parallel over batch, 4 samples per core, conv_w/conv_b
replicated (no collectives).

v25 (bf16): everything bf16 on the wire (tolerance 2e-2, bf16 end-to-end
measures ~5e-3). xk/x0 ship host-side zero-padded. SDMA engines drain
queues with strict per-engine priority (sync first), so streams are
ordered by need and each chunk that gates compute gets its own DMA (its
own completion semaphore):

  sync ring:   A=[xk_pad | wt p0-15] -> down0 -> up(k<4,h1) -> up(k>=4,h1) -> out0
  scalar ring: D1=[wt p16-31] -> D2=[x0_pad | bias] -> down1
               -> up(k<4,h0) -> up(k>=4,h0) -> out1

The W2 shuffle bounces through DRAM as a straight contiguous dump with the
permutation on the (slow, 256B-descriptor) readback side. The readback is
split 4 ways (k-half x h-half): the k<4 quarter-pair runs hidden under the
weight stream right after down0, and the k>=4 pair runs on both rings in
parallel. The last PSUM cast is split so only a 128-col cast gates down1.

Device mapping (per core, c = 4*b_loc + a in [0,16), t = 2p + q, p = 4k + j):
  step 1: 32 matmuls, one per t-pair p: stationary lhsT = zero-padded xk tile
          [K=128 (q,y), M=32 (q',c)], moving rhs = host-pre-transposed conv_w
          tile [128 (q,y), 128 n]. Four pairs (j=0..3) share a pass k via
          column tiling -> PSUM [32j+16q'+c, n] per pass.
  step 2: 8 matmuls, one per c-pair c2: stationary lhsT = zero-padded x0 tile
          [K=128 (h,t), M=128 (h',m)], moving rhs = shuffled W2 tile
          [128 (h,t), 128 n] -> PSUM [64h+m, n]; bias fused into the
          PSUM->SBUF cast on the vector engine.
"""

import ml_dtypes
import numpy as np

BS, TS, F, NF = 32, 64, 256, 128
NCORES = 8
B = BS // NCORES  # 4 local batches per core

F32 = np.float32
BF16 = ml_dtypes.bfloat16


# ---------------------------------------------------------------------------
# Host-side packing
# ---------------------------------------------------------------------------

def _pack_wt(conv_w: np.ndarray) -> np.ndarray:
    # WT[64q+y, 128p+n] = conv_w[n, 2p+q, y]
    wt = conv_w.transpose(1, 2, 0).reshape(32, 2, 64, NF)  # [p, q, y, n]
    wt = wt.transpose(1, 2, 0, 3)                          # [q, y, p, n]
    return np.ascontiguousarray(wt.reshape(128, 32 * NF)).astype(BF16)


def _pack_xk(xk_shard: np.ndarray) -> np.ndarray:
    # XK[64q+y, 32p+16q'+c] = xk[b, y, 8p+4q+a] iff q'==q else 0  (c = 4b+a)
    xq = xk_shard.reshape(B, TS, 32, 2, 4)       # [b, y, p, q, a]
    src = xq.transpose(3, 1, 2, 0, 4)            # [q, y, p, b, a]
    arr = np.zeros((2, TS, 32, 2, B, 4), dtype=F32)
    arr[0, :, :, 0] = src[0]
    arr[1, :, :, 1] = src[1]
    return arr.reshape(128, 1024).astype(BF16)


def _pack_x0(x0_shard: np.ndarray) -> np.ndarray:
    # X0L[64h+t, 128c2+64h'+m] = x0[b(c), m, 4t+a(c)] iff h'==h  (c = 2c2+h)
    xt = x0_shard.reshape(B, TS, TS, 4).transpose(0, 3, 2, 1)  # [b, a, t, m]
    flat = xt.reshape(16, TS, TS)                              # [c, t, m]
    arr = np.zeros((2, TS, 8, 2, TS), dtype=F32)               # [h, t, c2, h', m]
    for h in (0, 1):
        arr[h, :, :, h, :] = flat[2 * np.arange(8) + h].transpose(1, 0, 2)
    return arr.reshape(128, 1024).astype(BF16)


def _unpack_out(out_pack: np.ndarray, out_full: np.ndarray, r: int) -> None:
    # out_pack[64h+m, 128c2+n] = out[4r+b(c), n, a(c)*64+m], c = 2*c2+h
    o = np.asarray(out_pack).astype(F32).reshape(2, TS, 8, NF)  # [h, m, c2, n]
    for c2 in range(8):
        for h in (0, 1):
            c = 2 * c2 + h
            b, a = divmod(c, 4)
            out_full[4 * r + b, :, a * TS:(a + 1) * TS] = o[h, :, c2, :].T


# ---------------------------------------------------------------------------
# Device program
# ---------------------------------------------------------------------------

_prog_cache = {}


def _emit_body(nc, tc, pool, ps_pool, in_d, in2_d, out_d, w2b_d, version=25):
    import concourse.mybir as mybir

    bf = mybir.dt.bfloat16
    f32 = mybir.dt.float32

    # PE warm-up: back-to-back matmuls on a zeroed bf16 tile while the input
    # DMAs stream in; gets the HAM clock gate to 2.4GHz before step 1 starts.
    warm_s = pool.tile([128, 512], bf, tag="warm")
    nc.vector.memset(warm_s[:], 0.0)
    ps_w = ps_pool.tile([128, 512], f32, tag="warm_ps")
    for _ in range(6):
        nc.tensor.matmul(ps_w[:, :], warm_s[:, 0:128], warm_s[:, :],
                         start=True, stop=True)

    # input streams; each chunk that independently gates compute gets its
    # own DMA so its completion semaphore fires as early as possible
    in_s = pool.tile([128, 3072], bf, tag="in")     # [xk_pad | wt p0-15]
    nc.sync.dma_start(in_s[:], in_d.ap())
    wt1_s = pool.tile([128, 2048], bf, tag="wt1")   # wt p16-31
    nc.scalar.dma_start(wt1_s[:], in2_d.ap()[:, 0:2048])
    x0b_s = pool.tile([128, 1152], bf, tag="x0b")   # [x0_pad | bias]
    nc.scalar.dma_start(x0b_s[:], in2_d.ap()[:, 2048:3200])

    xk_pad = in_s[:, 0:1024]
    x0_pad = x0b_s[:, 0:1024]
    bias_s = x0b_s[:, 1024:1152]

    def wt_cols(p):  # rhs tile [128, 128] for pair p
        if p < 16:
            return in_s[:, 1024 + 128 * p:1024 + 128 * (p + 1)]
        return wt1_s[:, 128 * (p - 16):128 * (p - 15)]

    # ---- step 1: W2 = xk . wT, contract y (K = 128 = (q, y)) ----
    w2_s = pool.tile([128, 1024], bf, tag="w2")
    w2r_s = pool.tile([128, 1024], bf, tag="w2r")

    ps1_0 = ps_pool.tile([128, 512], f32, tag="s1_0")
    ps1_1 = ps_pool.tile([128, 512], f32, tag="s1_1")
    ps1s = [ps1_0, ps1_1]
    for u in range(2):
        ps1 = ps1s[u]
        for k in range(4 * u, 4 * u + 4):
            for j in range(4):
                p = 4 * k + j
                nc.tensor.matmul(
                    ps1[32 * j:32 * (j + 1), 128 * (k % 4):128 * (k % 4 + 1)],
                    xk_pad[:, 32 * p:32 * (p + 1)],
                    wt_cols(p),
                    start=True,
                    stop=True,
                    tile_position=(0, 32 * j),
                )
        # split the PSUM cast so only a thin 128-col cast gates the second
        # down DMA (the bulk cast overlaps the last pass's matmuls)
        if u == 0:
            nc.vector.tensor_copy(w2_s[:, 0:512], ps1[:, :])
        else:
            nc.vector.tensor_copy(w2_s[:, 512:896], ps1[:, 0:384])
            nc.vector.tensor_copy(w2_s[:, 896:1024], ps1[:, 384:512])
        # straight contiguous dump of this half (partition (j,q,c2,h),
        # free (k,n)); the permutation happens on the readback below
        (nc.sync if u == 0 else nc.scalar).dma_start(
            w2b_d.ap()[:, 512 * u:512 * (u + 1)],
            w2_s[:, 512 * u:512 * (u + 1)])

    # ---- shuffle readback: partition (j,q,c2,h) -> (h,k,j,q) = (h,t),
    # split 4 ways (k-half x h-half): the k<4 pair runs hidden under the
    # weight stream right after down0; each pair uses both rings ----
    src = w2b_d.ap().rearrange("(j q c2 h) (k n) -> h k j q c2 n",
                               j=4, q=2, c2=8, h=2, k=8, n=128)
    for u in range(2):
        for h in range(2):
            dst = w2r_s[64 * h + 32 * u:64 * h + 32 * u + 32, :].rearrange(
                "p (c2 n) -> p c2 n", c2=8)
            (nc.scalar if h == 0 else nc.sync).dma_start(
                dst, src[h, 4 * u:4 * (u + 1)])

    # ---- step 2: out = x0 . W2, contract t (K = 128 = (h, t)) ----
    out_s = pool.tile([128, 1024], bf, tag="out")
    bias4 = bias_s.unsqueeze(1).broadcast_to([128, 4, 128])
    for u in range(2):
        ps2 = ps_pool.tile([128, 512], f32, tag=f"s2_{u}")
        for c2 in range(4 * u, 4 * u + 4):
            nc.tensor.matmul(
                ps2[:, 128 * (c2 % 4):128 * (c2 % 4 + 1)],
                x0_pad[:, 128 * c2:128 * (c2 + 1)],
                w2r_s[:, 128 * c2:128 * (c2 + 1)],
                start=True,
                stop=True,
            )
        nc.vector.tensor_add(
            out_s[:, 512 * u:512 * (u + 1)].rearrange("p (f n) -> p f n", f=4),
            ps2[:, :].rearrange("p (f n) -> p f n", f=4),
            bias4,
        )
        (nc.sync if u == 0 else nc.scalar).dma_start(
            out_d.ap()[:, 512 * u:512 * (u + 1)],
            out_s[:, 512 * u:512 * (u + 1)])


def _build_program(version=25):
    if version in _prog_cache:
        return _prog_cache[version]

    from contextlib import ExitStack

    import concourse.bacc as bacc
    import concourse.mybir as mybir
    import concourse.tile as tile

    bf = mybir.dt.bfloat16
    nc = bacc.Bacc("TRN2", target_bir_lowering=False, debug=False)

    # in  = [xk_pad (1024) | wt pairs 0..15 (2048)]
    # in2 = [wt pairs 16..31 (2048) | x0_pad (1024) | bias (128)]
    in_d = nc.dram_tensor("in_pack", [128, 3072], bf, kind="ExternalInput")
    in2_d = nc.dram_tensor("in2_pack", [128, 3200], bf, kind="ExternalInput")
    out_d = nc.dram_tensor("out_pack", [128, 1024], bf, kind="ExternalOutput")
    # bounce buffer, straight dump of w2_s: [(j q c2 h), (k n)]
    w2b_d = nc.dram_tensor("w2_bounce", [128, 1024], bf)

    with tile.TileContext(nc) as tc, ExitStack() as ctx:
        pool = ctx.enter_context(tc.tile_pool(name="io", bufs=1))
        ps_pool = ctx.enter_context(tc.tile_pool(name="ps", bufs=1, space="PSUM"))
        _emit_body(nc, tc, pool, ps_pool, in_d, in2_d, out_d, w2b_d,
                   version=version)

    nc.compile()
    _prog_cache[version] = nc
    return nc


def pack_core_inputs(x_0, x_k, conv_w, conv_b, version=25):
    """Returns (in_maps list of 8 dicts) for run_bass_kernel_spmd."""
    wt = _pack_wt(np.asarray(conv_w, dtype=F32))
    bias = np.ascontiguousarray(
        np.broadcast_to(np.asarray(conv_b, dtype=F32), (128, 128))
    ).astype(BF16)
    x0 = np.asarray(x_0, dtype=F32)
    xk = np.asarray(x_k, dtype=F32)
    in_maps = []
    for r in range(NCORES):
        in_pack = np.concatenate(
            [_pack_xk(xk[B * r:B * (r + 1)]), wt[:, :2048]], axis=1)
        in2_pack = np.concatenate(
            [wt[:, 2048:], _pack_x0(x0[B * r:B * (r + 1)]), bias], axis=1)
        in_maps.append({
            "in_pack": np.ascontiguousarray(in_pack),
            "in2_pack": np.ascontiguousarray(in2_pack),
        })
    return in_maps


VERSION = 25  # current best variant


def kernel(x_0, x_k, conv_w, conv_b):
    from concourse.bass_utils import run_bass_kernel_spmd

    nc = _build_program(VERSION)
    in_maps = pack_core_inputs(x_0, x_k, conv_w, conv_b, version=VERSION)
    res = run_bass_kernel_spmd(nc, in_maps, core_ids=list(range(NCORES)))
    out = np.empty((BS, NF, F), dtype=F32)
    for r in range(NCORES):
        _unpack_out(res.results[r]["out_pack"], out, r)
    return out


# ---------------------------------------------------------------------------
# numpy model of the packed device program (for testing the packing logic)
# ---------------------------------------------------------------------------

def _pad_xk(xkd: np.ndarray) -> np.ndarray:
    arr = np.zeros((2, TS, 32, 2, 16), dtype=F32)  # [q, y, p, q', c]
    d = xkd.astype(F32).reshape(2, TS, 32, 16)     # [q, y, p, c]
    for q in (0, 1):
        arr[q, :, :, q] = d[q]
    return arr.reshape(128, 1024)


def _pad_x0(x0d: np.ndarray) -> np.ndarray:
    arr = np.zeros((2, TS, 8, 2, TS), dtype=F32)   # [h, t, c2, h', m]
    d = x0d.astype(F32).reshape(2, TS, 8, TS)      # [h, t, c2, m]
    for h in (0, 1):
        arr[h, :, :, h] = d[h]
    return arr.reshape(128, 1024)


def _numpy_model(x_0, x_k, conv_w, conv_b):
    out = np.empty((BS, NF, F), dtype=F32)
    in_maps = pack_core_inputs(x_0, x_k, conv_w, conv_b)
    for r in range(NCORES):
        m = in_maps[r]
        ip = m["in_pack"].astype(F32)
        ip2 = m["in2_pack"].astype(F32)
        xk_s = ip[:, :1024]
        wt = np.concatenate([ip[:, 1024:3072], ip2[:, :2048]], axis=1)
        x0l = ip2[:, 2048:3072]
        bias = ip2[:, 3072:3200]
        w2 = np.zeros((128, 1024), dtype=F32)
        for k in range(8):
            ps1 = np.zeros((128, 128), dtype=F32)
            for j in range(4):
                p = 4 * k + j
                ps1[32 * j:32 * (j + 1), :] = (
                    xk_s[:, 32 * p:32 * (p + 1)].T @ wt[:, 128 * p:128 * (p + 1)]
                )
            w2[:, 128 * k:128 * (k + 1)] = ps1
        w2 = w2.astype(BF16).astype(F32)  # PSUM->SBUF bf16 cast
        # bounce: straight dump, permuting readback -> partition (h,k,j,q)
        srcA = w2.reshape(4, 2, 8, 2, 8, 128)          # [j,q,c2,h,k,n]
        w2b = srcA.transpose(3, 4, 0, 1, 2, 5)         # [h,k,j,q,c2,n]
        w2r = w2b.reshape(128, 1024)
        out_pack = np.empty((128, 1024), dtype=F32)
        for c2 in range(8):
            out_pack[:, 128 * c2:128 * (c2 + 1)] = (
                x0l[:, 128 * c2:128 * (c2 + 1)].T @ w2r[:, 128 * c2:128 * (c2 + 1)]
                + bias
            )
        _unpack_out(out_pack.astype(BF16), out, r)
    return out


# revision 9
# speedup vs baseline: 1.0318x; 1.0052x over previous
"""Trainium2 Bass kernel for the CIN-style layer:

    z   = einsum('btf,byf->bfty', x_0, x_k)            # pairwise outer products
    z   = z.reshape(bs, ts0, f, tsk)                   # flat reinterpretation
    out = einsum('btiy,nty->bni', z, conv_w) + conv_b  # strided conv reduction

Shapes: x_0 (32, 64, 256), x_k (32, 64, 256), conv_w (128, 64, 64),
conv_b (128,) -> out (32, 128, 256).

Math: with i = a*64 + m  (a = i//64, m = i%64) and feature f = 4t + a the
reference reduces to a two-step factorization:

    W2[b,n,t,a]      = sum_y x_k[b,y,4t+a] * conv_w[n,t,y]         (contract y)
    out[b,n,a*64+m]  = sum_t x_0[b,m,4t+a] * W2[b,n,t,a] + conv_b  (contract t)

Sharding: pure data parallel over batch, 4 samples per core, conv_w/conv_b
replicated (no collectives).

v25 (bf16): everything bf16 on the wire (tolerance 2e-2, bf16 end-to-end
measures ~5e-3). xk/x0 ship host-side zero-padded. SDMA engines drain
queues with strict per-engine priority (sync first), so streams are
ordered by need and each chunk that gates compute gets its own DMA (its
own completion semaphore):

  sync ring:   A=[xk_pad | wt p0-15] -> down0 -> up(h0) -> up(h1) -> out0
  scalar ring: D1=[wt p16-31] -> D2=[x0_pad | bias] -> down1 -> out1

The W2 shuffle bounces through DRAM as a straight contiguous dump with the
permutation on the (slow, 256B-descriptor) readback side. The readback is
split into h-halves, which hit disjoint SDMA engine sets (even/odd ports)
and so drain concurrently even on one queue. The last PSUM cast is split
so only a thin 128-col cast gates down1.

Device mapping (per core, c = 4*b_loc + a in [0,16), t = 2p + q, p = 4k + j):
  step 1: 32 matmuls, one per t-pair p: stationary lhsT = zero-padded xk tile
          [K=128 (q,y), M=32 (q',c)], moving rhs = host-pre-transposed conv_w
          tile [128 (q,y), 128 n]. Four pairs (j=0..3) share a pass k via
          column tiling -> PSUM [32j+16q'+c, n] per pass.
  step 2: 8 matmuls, one per c-pair c2: stationary lhsT = zero-padded x0 tile
          [K=128 (h,t), M=128 (h',m)], moving rhs = shuffled W2 tile
          [128 (h,t), 128 n] -> PSUM [64h+m, n]; bias fused into the
          PSUM->SBUF cast on the vector engine.
"""

import ml_dtypes
import numpy as np

BS, TS, F, NF = 32, 64, 256, 128
NCORES = 8
B = BS // NCORES  # 4 local batches per core

F32 = np.float32
BF16 = ml_dtypes.bfloat16


# ---------------------------------------------------------------------------
# Host-side packing
# ---------------------------------------------------------------------------

def _pack_wt(conv_w: np.ndarray) -> np.ndarray:
    # WT[64q+y, 128p+n] = conv_w[n, 2p+q, y]
    wt = conv_w.transpose(1, 2, 0).reshape(32, 2, 64, NF)  # [p, q, y, n]
    wt = wt.transpose(1, 2, 0, 3)                          # [q, y, p, n]
    return np.ascontiguousarray(wt.reshape(128, 32 * NF)).astype(BF16)


def _pack_xk(xk_shard: np.ndarray) -> np.ndarray:
    # XK[64q+y, 32p+16q'+c] = xk[b, y, 8p+4q+a] iff q'==q else 0  (c = 4b+a)
    xq = xk_shard.reshape(B, TS, 32, 2, 4)       # [b, y, p, q, a]
    src = xq.transpose(3, 1, 2, 0, 4)            # [q, y, p, b, a]
    arr = np.zeros((2, TS, 32, 2, B, 4), dtype=F32)
    arr[0, :, :, 0] = src[0]
    arr[1, :, :, 1] = src[1]
    return arr.reshape(128, 1024).astype(BF16)


def _pack_x0(x0_shard: np.ndarray) -> np.ndarray:
    # X0L[64h+t, 128c2+64h'+m] = x0[b(c), m, 4t+a(c)] iff h'==h  (c = 2c2+h)
    xt = x0_shard.reshape(B, TS, TS, 4).transpose(0, 3, 2, 1)  # [b, a, t, m]
    flat = xt.reshape(16, TS, TS)                              # [c, t, m]
    arr = np.zeros((2, TS, 8, 2, TS), dtype=F32)               # [h, t, c2, h', m]
    for h in (0, 1):
        arr[h, :, :, h, :] = flat[2 * np.arange(8) + h].transpose(1, 0, 2)
    return arr.reshape(128, 1024).astype(BF16)


def _unpack_out(out_pack: np.ndarray, out_full: np.ndarray, r: int) -> None:
    # out_pack[64h+m, 128c2+n] = out[4r+b(c), n, a(c)*64+m], c = 2*c2+h
    o = np.asarray(out_pack).astype(F32).reshape(2, TS, 8, NF)  # [h, m, c2, n]
    for c2 in range(8):
        for h in (0, 1):
            c = 2 * c2 + h
            b, a = divmod(c, 4)
            out_full[4 * r + b, :, a * TS:(a + 1) * TS] = o[h, :, c2, :].T


# ---------------------------------------------------------------------------
# Device program
# ---------------------------------------------------------------------------

_prog_cache = {}


def _emit_body(nc, tc, pool, ps_pool, in_d, in2_d, out_d, w2b_d, version=26):
    import concourse.mybir as mybir

    bf = mybir.dt.bfloat16
    f32 = mybir.dt.float32

    # PE warm-up: back-to-back matmuls on a zeroed bf16 tile while the input
    # DMAs stream in; gets the HAM clock gate to 2.4GHz before step 1 starts.
    warm_s = pool.tile([128, 512], bf, tag="warm")
    nc.vector.memset(warm_s[:], 0.0)
    ps_w = ps_pool.tile([128, 512], f32, tag="warm_ps")
    for _ in range(6):
        nc.tensor.matmul(ps_w[:, :], warm_s[:, 0:128], warm_s[:, :],
                         start=True, stop=True)

    # input streams; each chunk that independently gates compute gets its
    # own DMA so its completion semaphore fires as early as possible
    in_s = pool.tile([128, 3072], bf, tag="in")     # [xk_pad | wt p0-15]
    nc.sync.dma_start(in_s[:], in_d.ap())
    wt1_s = pool.tile([128, 2048], bf, tag="wt1")   # wt p16-31
    nc.scalar.dma_start(wt1_s[:], in2_d.ap()[:, 0:2048])
    x0b_s = pool.tile([128, 1152], bf, tag="x0b")   # [x0_pad | bias]
    nc.scalar.dma_start(x0b_s[:], in2_d.ap()[:, 2048:3200])

    xk_pad = in_s[:, 0:1024]
    x0_pad = x0b_s[:, 0:1024]
    bias_s = x0b_s[:, 1024:1152]

    def wt_cols(p):  # rhs tile [128, 128] for pair p
        if p < 16:
            return in_s[:, 1024 + 128 * p:1024 + 128 * (p + 1)]
        return wt1_s[:, 128 * (p - 16):128 * (p - 15)]

    # ---- step 1: W2 = xk . wT, contract y (K = 128 = (q, y)) ----
    w2_s = pool.tile([128, 1024], bf, tag="w2")
    w2r_s = pool.tile([128, 1024], bf, tag="w2r")

    ps1_0 = ps_pool.tile([128, 512], f32, tag="s1_0")
    ps1_1 = ps_pool.tile([128, 512], f32, tag="s1_1")
    ps1s = [ps1_0, ps1_1]
    for u in range(2):
        ps1 = ps1s[u]
        for k in range(4 * u, 4 * u + 4):
            for j in range(4):
                p = 4 * k + j
                nc.tensor.matmul(
                    ps1[32 * j:32 * (j + 1), 128 * (k % 4):128 * (k % 4 + 1)],
                    xk_pad[:, 32 * p:32 * (p + 1)],
                    wt_cols(p),
                    start=True,
                    stop=True,
                    tile_position=(0, 32 * j),
                )
        # split the PSUM cast so only a thin 128-col cast gates the second
        # down DMA (the bulk cast overlaps the last pass's matmuls)
        if u == 0:
            nc.vector.tensor_copy(w2_s[:, 0:512], ps1[:, :])
        else:
            nc.vector.tensor_copy(w2_s[:, 512:896], ps1[:, 0:384])
            nc.vector.tensor_copy(w2_s[:, 896:1024], ps1[:, 384:512])
        # straight contiguous dump of this half (partition (j,q,c2,h),
        # free (k,n)); the permutation happens on the readback below
        (nc.sync if u == 0 else nc.scalar).dma_start(
            w2b_d.ap()[:, 512 * u:512 * (u + 1)],
            w2_s[:, 512 * u:512 * (u + 1)])

    # ---- shuffle readback: partition (j,q,c2,h) -> (h,k,j,q) = (h,t),
    # split into h-halves. The halves hit disjoint SDMA engine sets
    # (even/odd ports), so both run concurrently even on one queue; both
    # go on sync so the scalar ring stays free for out1's issue ----
    src = w2b_d.ap().rearrange("(j q c2 h) (k n) -> h k j q c2 n",
                               j=4, q=2, c2=8, h=2, k=8, n=128)
    for h in range(2):
        dst = w2r_s[64 * h:64 * (h + 1), :].rearrange(
            "p (c2 n) -> p c2 n", c2=8)
        nc.sync.dma_start(dst, src[h])

    # ---- step 2: out = x0 . W2, contract t (K = 128 = (h, t)) ----
    out_s = pool.tile([128, 1024], bf, tag="out")
    bias4 = bias_s.unsqueeze(1).broadcast_to([128, 4, 128])
    for u in range(2):
        ps2 = ps_pool.tile([128, 512], f32, tag=f"s2_{u}")
        for c2 in range(4 * u, 4 * u + 4):
            nc.tensor.matmul(
                ps2[:, 128 * (c2 % 4):128 * (c2 % 4 + 1)],
                x0_pad[:, 128 * c2:128 * (c2 + 1)],
                w2r_s[:, 128 * c2:128 * (c2 + 1)],
                start=True,
                stop=True,
            )
        nc.vector.tensor_add(
            out_s[:, 512 * u:512 * (u + 1)].rearrange("p (f n) -> p f n", f=4),
            ps2[:, :].rearrange("p (f n) -> p f n", f=4),
            bias4,
        )
        (nc.sync if u == 0 else nc.scalar).dma_start(
            out_d.ap()[:, 512 * u:512 * (u + 1)],
            out_s[:, 512 * u:512 * (u + 1)])


def _build_program(version=26):
    if version in _prog_cache:
        return _prog_cache[version]

    from contextlib import ExitStack

    import concourse.bacc as bacc
    import concourse.mybir as mybir
    import concourse.tile as tile

    bf = mybir.dt.bfloat16
    nc = bacc.Bacc("TRN2", target_bir_lowering=False, debug=False)

    # in  = [xk_pad (1024) | wt pairs 0..15 (2048)]
    # in2 = [wt pairs 16..31 (2048) | x0_pad (1024) | bias (128)]
    in_d = nc.dram_tensor("in_pack", [128, 3072], bf, kind="ExternalInput")
    in2_d = nc.dram_tensor("in2_pack", [128, 3200], bf, kind="ExternalInput")
    out_d = nc.dram_tensor("out_pack", [128, 1024], bf, kind="ExternalOutput")
    # bounce buffer, straight dump of w2_s: [(j q c2 h), (k n)]
    w2b_d = nc.dram_tensor("w2_bounce", [128, 1024], bf)

    with tile.TileContext(nc) as tc, ExitStack() as ctx:
        pool = ctx.enter_context(tc.tile_pool(name="io", bufs=1))
        ps_pool = ctx.enter_context(tc.tile_pool(name="ps", bufs=1, space="PSUM"))
        _emit_body(nc, tc, pool, ps_pool, in_d, in2_d, out_d, w2b_d,
                   version=version)

    nc.compile()
    _prog_cache[version] = nc
    return nc


def pack_core_inputs(x_0, x_k, conv_w, conv_b, version=26):
    """Returns (in_maps list of 8 dicts) for run_bass_kernel_spmd."""
    wt = _pack_wt(np.asarray(conv_w, dtype=F32))
    bias = np.ascontiguousarray(
        np.broadcast_to(np.asarray(conv_b, dtype=F32), (128, 128))
    ).astype(BF16)
    x0 = np.asarray(x_0, dtype=F32)
    xk = np.asarray(x_k, dtype=F32)
    in_maps = []
    for r in range(NCORES):
        in_pack = np.concatenate(
            [_pack_xk(xk[B * r:B * (r + 1)]), wt[:, :2048]], axis=1)
        in2_pack = np.concatenate(
            [wt[:, 2048:], _pack_x0(x0[B * r:B * (r + 1)]), bias], axis=1)
        in_maps.append({
            "in_pack": np.ascontiguousarray(in_pack),
            "in2_pack": np.ascontiguousarray(in2_pack),
        })
    return in_maps


VERSION = 26  # current best variant


def kernel(x_0, x_k, conv_w, conv_b):
    from concourse.bass_utils import run_bass_kernel_spmd

    nc = _build_program(VERSION)
    in_maps = pack_core_inputs(x_0, x_k, conv_w, conv_b, version=VERSION)
    res = run_bass_kernel_spmd(nc, in_maps, core_ids=list(range(NCORES)))
    out = np.empty((BS, NF, F), dtype=F32)
    for r in range(NCORES):
        _unpack_out(res.results[r]["out_pack"], out, r)
    return out


# ---------------------------------------------------------------------------
# numpy model of the packed device program (for testing the packing logic)
# ---------------------------------------------------------------------------

def _pad_xk(xkd: np.ndarray) -> np.ndarray:
    arr = np.zeros((2, TS, 32, 2, 16), dtype=F32)  # [q, y, p, q', c]
    d = xkd.astype(F32).reshape(2, TS, 32, 16)     # [q, y, p, c]
    for q in (0, 1):
        arr[q, :, :, q] = d[q]
    return arr.reshape(128, 1024)


def _pad_x0(x0d: np.ndarray) -> np.ndarray:
    arr = np.zeros((2, TS, 8, 2, TS), dtype=F32)   # [h, t, c2, h', m]
    d = x0d.astype(F32).reshape(2, TS, 8, TS)      # [h, t, c2, m]
    for h in (0, 1):
        arr[h, :, :, h] = d[h]
    return arr.reshape(128, 1024)


def _numpy_model(x_0, x_k, conv_w, conv_b):
    out = np.empty((BS, NF, F), dtype=F32)
    in_maps = pack_core_inputs(x_0, x_k, conv_w, conv_b)
    for r in range(NCORES):
        m = in_maps[r]
        ip = m["in_pack"].astype(F32)
        ip2 = m["in2_pack"].astype(F32)
        xk_s = ip[:, :1024]
        wt = np.concatenate([ip[:, 1024:3072], ip2[:, :2048]], axis=1)
        x0l = ip2[:, 2048:3072]
        bias = ip2[:, 3072:3200]
        w2 = np.zeros((128, 1024), dtype=F32)
        for k in range(8):
            ps1 = np.zeros((128, 128), dtype=F32)
            for j in range(4):
                p = 4 * k + j
                ps1[32 * j:32 * (j + 1), :] = (
                    xk_s[:, 32 * p:32 * (p + 1)].T @ wt[:, 128 * p:128 * (p + 1)]
                )
            w2[:, 128 * k:128 * (k + 1)] = ps1
        w2 = w2.astype(BF16).astype(F32)  # PSUM->SBUF bf16 cast
        # bounce: straight dump, permuting readback -> partition (h,k,j,q)
        srcA = w2.reshape(4, 2, 8, 2, 8, 128)          # [j,q,c2,h,k,n]
        w2b = srcA.transpose(3, 4, 0, 1, 2, 5)         # [h,k,j,q,c2,n]
        w2r = w2b.reshape(128, 1024)
        out_pack = np.empty((128, 1024), dtype=F32)
        for c2 in range(8):
            out_pack[:, 128 * c2:128 * (c2 + 1)] = (
                x0l[:, 128 * c2:128 * (c2 + 1)].T @ w2r[:, 128 * c2:128 * (c2 + 1)]
                + bias
            )
        _unpack_out(out_pack.astype(BF16), out, r)
    return out


# revision 10
# speedup vs baseline: 1.0672x; 1.0343x over previous
"""Trainium2 Bass kernel for the CIN-style layer:

    z   = einsum('btf,byf->bfty', x_0, x_k)            # pairwise outer products
    z   = z.reshape(bs, ts0, f, tsk)                   # flat reinterpretation
    out = einsum('btiy,nty->bni', z, conv_w) + conv_b  # strided conv reduction

Shapes: x_0 (32, 64, 256), x_k (32, 64, 256), conv_w (128, 64, 64),
conv_b (128,) -> out (32, 128, 256).

Math: with i = a*64 + m  (a = i//64, m = i%64) and feature f = 4t + a the
reference reduces to a two-step factorization:

    W2[b,n,t,a]      = sum_y x_k[b,y,4t+a] * conv_w[n,t,y]         (contract y)
    out[b,n,a*64+m]  = sum_t x_0[b,m,4t+a] * W2[b,n,t,a] + conv_b  (contract t)

Sharding: pure data parallel over batch, 4 samples per core, conv_w/conv_b
replicated (no collectives).

v27 (bf16, contiguous bounce): everything bf16 on the wire (tolerance
2e-2, bf16 end-to-end measures ~5e-3). Streams ordered by need across the
two strict-priority HWDGE queues:

  sync ring:   A=[xk_pad | wt p0-15] -> down0 -> up -> out0
  scalar ring: D1=[wt p16-31] -> D2=[x0_pad | bias] -> down1 -> out1

The W2 partition shuffle (step-1 partition (j,q,c2,h) -> step-2 partition
(h,t)) goes through a DRAM bounce laid out L[k, p, n] (k = pass, p =
step-1 partition, n). With the c-bits split as h = c>>3, c2 = c&7 and
step-2 rows ordered p' = 16k+4j+2q+h = 2t+h, BOTH bounce directions are
cheap: each down half is a 3-dim AP whose per-engine runs concatenate
into >=1KB writes, and the readback is a single fully-contiguous
[128, 1024] DMA (2KB rows, all 16 engines) instead of the 2048x256B
gather that previously cost 2.3us. The last PSUM cast is split so only a
thin 128-col cast gates down1.

Device mapping (per core, c = 4*b_loc + a in [0,16), t = 2p + q, p = 4k + j):
  step 1: 32 matmuls, one per t-pair p: stationary lhsT = zero-padded xk tile
          [K=128 (q,y), M=32 (q',c)], moving rhs = host-pre-transposed conv_w
          tile [128 (q,y), 128 n]. Four pairs (j=0..3) share a pass k via
          column tiling -> PSUM [32j+16q'+c, n] per pass.
  step 2: 8 matmuls, one per c-pair c2: stationary lhsT = zero-padded x0 tile
          [K=128 (2t+h), M=128 (h',m)], moving rhs = shuffled W2 tile
          [128 (2t+h), 128 n] -> PSUM [64h'+m, n]; bias fused into the
          PSUM->SBUF cast on the vector engine.
"""

import ml_dtypes
import numpy as np

BS, TS, F, NF = 32, 64, 256, 128
NCORES = 8
B = BS // NCORES  # 4 local batches per core

F32 = np.float32
BF16 = ml_dtypes.bfloat16


# ---------------------------------------------------------------------------
# Host-side packing
# ---------------------------------------------------------------------------

def _pack_wt(conv_w: np.ndarray) -> np.ndarray:
    # WT[64q+y, 128p+n] = conv_w[n, 2p+q, y]
    wt = conv_w.transpose(1, 2, 0).reshape(32, 2, 64, NF)  # [p, q, y, n]
    wt = wt.transpose(1, 2, 0, 3)                          # [q, y, p, n]
    return np.ascontiguousarray(wt.reshape(128, 32 * NF)).astype(BF16)


def _pack_xk(xk_shard: np.ndarray) -> np.ndarray:
    # XK[64q+y, 32p+16q'+c] = xk[b, y, 8p+4q+a] iff q'==q else 0  (c = 4b+a)
    xq = xk_shard.reshape(B, TS, 32, 2, 4)       # [b, y, p, q, a]
    src = xq.transpose(3, 1, 2, 0, 4)            # [q, y, p, b, a]
    arr = np.zeros((2, TS, 32, 2, B, 4), dtype=F32)
    arr[0, :, :, 0] = src[0]
    arr[1, :, :, 1] = src[1]
    return arr.reshape(128, 1024).astype(BF16)


def _pack_x0(x0_shard: np.ndarray) -> np.ndarray:
    # X0L[2t+h, 128c2+64h'+m] = x0[b(c), m, 4t+a(c)] iff h'==h  (c = 8h+c2)
    xt = x0_shard.reshape(B, TS, TS, 4).transpose(0, 3, 2, 1)  # [b, a, t, m]
    flat = xt.reshape(16, TS, TS)                              # [c, t, m]
    arr = np.zeros((TS, 2, 8, 2, TS), dtype=F32)               # [t, h, c2, h', m]
    for h in (0, 1):
        arr[:, h, :, h, :] = flat[8 * h + np.arange(8)].transpose(1, 0, 2)
    return arr.reshape(128, 8 * 128).astype(BF16)


def _unpack_out(out_pack: np.ndarray, out_full: np.ndarray, r: int) -> None:
    # out_pack[64h+m, 128c2+n] = out[4r+b(c), n, a(c)*64+m], c = 8h+c2
    o = np.asarray(out_pack).astype(F32).reshape(2, TS, 8, NF)  # [h, m, c2, n]
    for c2 in range(8):
        for h in (0, 1):
            c = 8 * h + c2
            b, a = divmod(c, 4)
            out_full[4 * r + b, :, a * TS:(a + 1) * TS] = o[h, :, c2, :].T


# ---------------------------------------------------------------------------
# Device program
# ---------------------------------------------------------------------------

_prog_cache = {}


def _emit_body(nc, tc, pool, ps_pool, in_d, in2_d, out_d, w2b_d, version=27):
    import concourse.mybir as mybir

    bf = mybir.dt.bfloat16
    f32 = mybir.dt.float32

    # PE warm-up: back-to-back matmuls on a zeroed bf16 tile while the input
    # DMAs stream in; gets the HAM clock gate to 2.4GHz before step 1 starts.
    warm_s = pool.tile([128, 512], bf, tag="warm")
    nc.vector.memset(warm_s[:], 0.0)
    ps_w = ps_pool.tile([128, 512], f32, tag="warm_ps")
    for _ in range(6):
        nc.tensor.matmul(ps_w[:, :], warm_s[:, 0:128], warm_s[:, :],
                         start=True, stop=True)

    # input streams; each chunk that independently gates compute gets its
    # own DMA so its completion semaphore fires as early as possible
    in_s = pool.tile([128, 3072], bf, tag="in")     # [xk_pad | wt p0-15]
    nc.sync.dma_start(in_s[:], in_d.ap())
    wt1_s = pool.tile([128, 2048], bf, tag="wt1")   # wt p16-31
    nc.scalar.dma_start(wt1_s[:], in2_d.ap()[:, 0:2048])
    x0b_s = pool.tile([128, 1152], bf, tag="x0b")   # [x0_pad | bias]
    nc.scalar.dma_start(x0b_s[:], in2_d.ap()[:, 2048:3200])

    xk_pad = in_s[:, 0:1024]
    x0_pad = x0b_s[:, 0:1024]
    bias_s = x0b_s[:, 1024:1152]

    def wt_cols(p):  # rhs tile [128, 128] for pair p
        if p < 16:
            return in_s[:, 1024 + 128 * p:1024 + 128 * (p + 1)]
        return wt1_s[:, 128 * (p - 16):128 * (p - 15)]

    # ---- step 1: W2 = xk . wT, contract y (K = 128 = (q, y)) ----
    w2_s = pool.tile([128, 1024], bf, tag="w2")
    w2r_s = pool.tile([128, 1024], bf, tag="w2r")

    ps1_0 = ps_pool.tile([128, 512], f32, tag="s1_0")
    ps1_1 = ps_pool.tile([128, 512], f32, tag="s1_1")
    ps1s = [ps1_0, ps1_1]
    for u in range(2):
        ps1 = ps1s[u]
        for k in range(4 * u, 4 * u + 4):
            for j in range(4):
                p = 4 * k + j
                nc.tensor.matmul(
                    ps1[32 * j:32 * (j + 1), 128 * (k % 4):128 * (k % 4 + 1)],
                    xk_pad[:, 32 * p:32 * (p + 1)],
                    wt_cols(p),
                    start=True,
                    stop=True,
                    tile_position=(0, 32 * j),
                )
        # split the PSUM cast so only a thin 128-col cast gates the second
        # down DMA (the bulk cast overlaps the last pass's matmuls)
        if u == 0:
            nc.vector.tensor_copy(w2_s[:, 0:512], ps1[:, :])
        else:
            nc.vector.tensor_copy(w2_s[:, 512:896], ps1[:, 0:384])
            nc.vector.tensor_copy(w2_s[:, 896:1024], ps1[:, 384:512])
        # bounce down: L[k, p, n] = w2_s[p, 128k+n]. Per (engine, k) the
        # four consecutive partitions' 256B runs concatenate into 1KB
        # contiguous writes, so this runs near line rate.
        dst = w2b_d.ap()[4 * u:4 * (u + 1)].rearrange("k p n -> p k n")
        src = w2_s[:, 512 * u:512 * (u + 1)].rearrange("p (k n) -> p k n", k=4)
        (nc.sync if u == 0 else nc.scalar).dma_start(dst, src)

    # ---- shuffle readback: with rows p' = 16k+4j+2q+h = 2t+h and the
    # L[k, p, n] bounce layout, the readback is a single fully-contiguous
    # [128, 1024] read (2KB rows, all 16 engines) ----
    up_src = w2b_d.ap().rearrange("k (ph pl) n -> (k ph) (pl n)", ph=16, pl=8)
    nc.sync.dma_start(w2r_s[:], up_src)

    # ---- step 2: out = x0 . W2, contract t (K = 128 = (2t+h)) ----
    out_s = pool.tile([128, 1024], bf, tag="out")
    bias4 = bias_s.unsqueeze(1).broadcast_to([128, 4, 128])
    for u in range(2):
        ps2 = ps_pool.tile([128, 512], f32, tag=f"s2_{u}")
        for c2 in range(4 * u, 4 * u + 4):
            nc.tensor.matmul(
                ps2[:, 128 * (c2 % 4):128 * (c2 % 4 + 1)],
                x0_pad[:, 128 * c2:128 * (c2 + 1)],
                w2r_s[:, 128 * c2:128 * (c2 + 1)],
                start=True,
                stop=True,
            )
        nc.vector.tensor_add(
            out_s[:, 512 * u:512 * (u + 1)].rearrange("p (f n) -> p f n", f=4),
            ps2[:, :].rearrange("p (f n) -> p f n", f=4),
            bias4,
        )
        (nc.sync if u == 0 else nc.scalar).dma_start(
            out_d.ap()[:, 512 * u:512 * (u + 1)],
            out_s[:, 512 * u:512 * (u + 1)])


def _build_program(version=27):
    if version in _prog_cache:
        return _prog_cache[version]

    from contextlib import ExitStack

    import concourse.bacc as bacc
    import concourse.mybir as mybir
    import concourse.tile as tile

    bf = mybir.dt.bfloat16
    nc = bacc.Bacc("TRN2", target_bir_lowering=False, debug=False)

    # in  = [xk_pad (1024) | wt pairs 0..15 (2048)]
    # in2 = [wt pairs 16..31 (2048) | x0_pad (1024) | bias (128)]
    in_d = nc.dram_tensor("in_pack", [128, 3072], bf, kind="ExternalInput")
    in2_d = nc.dram_tensor("in2_pack", [128, 3200], bf, kind="ExternalInput")
    out_d = nc.dram_tensor("out_pack", [128, 1024], bf, kind="ExternalOutput")
    # bounce buffer: L[k, p, n] = w2_s[p, 128k+n]
    w2b_d = nc.dram_tensor("w2_bounce", [8, 128, 128], bf)

    with tile.TileContext(nc) as tc, ExitStack() as ctx:
        pool = ctx.enter_context(tc.tile_pool(name="io", bufs=1))
        ps_pool = ctx.enter_context(tc.tile_pool(name="ps", bufs=1, space="PSUM"))
        _emit_body(nc, tc, pool, ps_pool, in_d, in2_d, out_d, w2b_d,
                   version=version)

    nc.compile()
    _prog_cache[version] = nc
    return nc


def pack_core_inputs(x_0, x_k, conv_w, conv_b, version=27):
    """Returns (in_maps list of 8 dicts) for run_bass_kernel_spmd."""
    wt = _pack_wt(np.asarray(conv_w, dtype=F32))
    bias = np.ascontiguousarray(
        np.broadcast_to(np.asarray(conv_b, dtype=F32), (128, 128))
    ).astype(BF16)
    x0 = np.asarray(x_0, dtype=F32)
    xk = np.asarray(x_k, dtype=F32)
    in_maps = []
    for r in range(NCORES):
        in_pack = np.concatenate(
            [_pack_xk(xk[B * r:B * (r + 1)]), wt[:, :2048]], axis=1)
        in2_pack = np.concatenate(
            [wt[:, 2048:], _pack_x0(x0[B * r:B * (r + 1)]), bias], axis=1)
        in_maps.append({
            "in_pack": np.ascontiguousarray(in_pack),
            "in2_pack": np.ascontiguousarray(in2_pack),
        })
    return in_maps


VERSION = 27  # current best variant


def kernel(x_0, x_k, conv_w, conv_b):
    from concourse.bass_utils import run_bass_kernel_spmd

    nc = _build_program(VERSION)
    in_maps = pack_core_inputs(x_0, x_k, conv_w, conv_b, version=VERSION)
    res = run_bass_kernel_spmd(nc, in_maps, core_ids=list(range(NCORES)))
    out = np.empty((BS, NF, F), dtype=F32)
    for r in range(NCORES):
        _unpack_out(res.results[r]["out_pack"], out, r)
    return out


# ---------------------------------------------------------------------------
# numpy model of the packed device program (for testing the packing logic)
# ---------------------------------------------------------------------------

def _numpy_model(x_0, x_k, conv_w, conv_b):
    out = np.empty((BS, NF, F), dtype=F32)
    in_maps = pack_core_inputs(x_0, x_k, conv_w, conv_b)
    for r in range(NCORES):
        m = in_maps[r]
        ip = m["in_pack"].astype(F32)
        ip2 = m["in2_pack"].astype(F32)
        xk_s = ip[:, :1024]
        wt = np.concatenate([ip[:, 1024:3072], ip2[:, :2048]], axis=1)
        x0l = ip2[:, 2048:3072]
        bias = ip2[:, 3072:3200]
        w2 = np.zeros((128, 1024), dtype=F32)
        for k in range(8):
            ps1 = np.zeros((128, 128), dtype=F32)
            for j in range(4):
                p = 4 * k + j
                ps1[32 * j:32 * (j + 1), :] = (
                    xk_s[:, 32 * p:32 * (p + 1)].T @ wt[:, 128 * p:128 * (p + 1)]
                )
            w2[:, 128 * k:128 * (k + 1)] = ps1
        w2 = w2.astype(BF16).astype(F32)  # PSUM->SBUF bf16 cast
        # bounce L[k, p, n] = w2[p, 128k+n]; readback row p' = (k, p>>3),
        # col (p&7, n)
        L = w2.reshape(128, 8, 128).transpose(1, 0, 2)   # [k, p, n]
        w2r = np.ascontiguousarray(L).reshape(128, 1024)
        out_pack = np.empty((128, 1024), dtype=F32)
        for c2 in range(8):
            out_pack[:, 128 * c2:128 * (c2 + 1)] = (
                x0l[:, 128 * c2:128 * (c2 + 1)].T @ w2r[:, 128 * c2:128 * (c2 + 1)]
                + bias
            )
        _unpack_out(out_pack.astype(BF16), out, r)
    return out


# revision 11
# speedup vs baseline: 1.1202x; 1.0497x over previous
"""Trainium2 Bass kernel for the CIN-style layer:

    z   = einsum('btf,byf->bfty', x_0, x_k)            # pairwise outer products
    z   = z.reshape(bs, ts0, f, tsk)                   # flat reinterpretation
    out = einsum('btiy,nty->bni', z, conv_w) + conv_b  # strided conv reduction

Shapes: x_0 (32, 64, 256), x_k (32, 64, 256), conv_w (128, 64, 64),
conv_b (128,) -> out (32, 128, 256).

Math: with i = a*64 + m  (a = i//64, m = i%64) and feature f = 4t + a the
reference reduces to a two-step factorization:

    W2[b,n,t,a]      = sum_y x_k[b,y,4t+a] * conv_w[n,t,y]         (contract y)
    out[b,n,a*64+m]  = sum_t x_0[b,m,4t+a] * W2[b,n,t,a] + conv_b  (contract t)

Sharding: pure data parallel over batch, 4 samples per core, conv_w/conv_b
replicated (no collectives).

v27 (bf16, contiguous bounce): everything bf16 on the wire (tolerance
2e-2, bf16 end-to-end measures ~5e-3). Streams ordered by need across the
two strict-priority HWDGE queues:

  sync ring:   A=[xk_pad | wt p0-15] -> down0 -> up0 -> out0
  scalar ring: D1=[wt p16-31] -> D2=[x0_pad | bias] -> down1 -> up1 -> out1

The W2 partition shuffle (step-1 partition (j,q,c2,h) -> step-2 partition
(h,t)) goes through a DRAM bounce laid out L[k, p, n] (k = pass, p =
step-1 partition, n). With the c-bits split as h = c>>3, c2 = c&7 and
step-2 rows ordered p' = 16k+4j+2q+h = 2t+h, BOTH bounce directions are
cheap: each down half is a 3-dim AP whose per-engine runs concatenate
into >=1KB writes, and the readback is a single fully-contiguous
[128, 1024] DMA (2KB rows, all 16 engines) instead of the 2048x256B
gather that previously cost 2.3us. The last PSUM cast is split so only a
thin 128-col cast gates down1.

Device mapping (per core, c = 4*b_loc + a in [0,16), t = 2p + q, p = 4k + j):
  step 1: 32 matmuls, one per t-pair p: stationary lhsT = zero-padded xk tile
          [K=128 (q,y), M=32 (q',c)], moving rhs = host-pre-transposed conv_w
          tile [128 (q,y), 128 n]. Four pairs (j=0..3) share a pass k via
          column tiling -> PSUM [32j+16q'+c, n] per pass.
  step 2: 8 matmuls, one per c-pair c2: stationary lhsT = zero-padded x0 tile
          [K=128 (2t+h), M=128 (h',m)], moving rhs = shuffled W2 tile
          [128 (2t+h), 128 n] -> PSUM [64h'+m, n]; bias fused into the
          PSUM->SBUF cast on the vector engine.
"""

import ml_dtypes
import numpy as np

BS, TS, F, NF = 32, 64, 256, 128
NCORES = 8
B = BS // NCORES  # 4 local batches per core

F32 = np.float32
BF16 = ml_dtypes.bfloat16


# ---------------------------------------------------------------------------
# Host-side packing
# ---------------------------------------------------------------------------

def _pack_wt(conv_w: np.ndarray) -> np.ndarray:
    # WT[64q+y, 128p+n] = conv_w[n, 2p+q, y]
    wt = conv_w.transpose(1, 2, 0).reshape(32, 2, 64, NF)  # [p, q, y, n]
    wt = wt.transpose(1, 2, 0, 3)                          # [q, y, p, n]
    return np.ascontiguousarray(wt.reshape(128, 32 * NF)).astype(BF16)


def _pack_xk(xk_shard: np.ndarray) -> np.ndarray:
    # XK[64q+y, 32p+16q'+c] = xk[b, y, 8p+4q+a] iff q'==q else 0  (c = 4b+a)
    xq = xk_shard.reshape(B, TS, 32, 2, 4)       # [b, y, p, q, a]
    src = xq.transpose(3, 1, 2, 0, 4)            # [q, y, p, b, a]
    arr = np.zeros((2, TS, 32, 2, B, 4), dtype=F32)
    arr[0, :, :, 0] = src[0]
    arr[1, :, :, 1] = src[1]
    return arr.reshape(128, 1024).astype(BF16)


def _pack_x0(x0_shard: np.ndarray) -> np.ndarray:
    # X0L[2t+h, 128c2+64h'+m] = x0[b(c), m, 4t+a(c)] iff h'==h  (c = 8h+c2)
    xt = x0_shard.reshape(B, TS, TS, 4).transpose(0, 3, 2, 1)  # [b, a, t, m]
    flat = xt.reshape(16, TS, TS)                              # [c, t, m]
    arr = np.zeros((TS, 2, 8, 2, TS), dtype=F32)               # [t, h, c2, h', m]
    for h in (0, 1):
        arr[:, h, :, h, :] = flat[8 * h + np.arange(8)].transpose(1, 0, 2)
    return arr.reshape(128, 8 * 128).astype(BF16)


def _unpack_out(out_pack: np.ndarray, out_full: np.ndarray, r: int) -> None:
    # out_pack[64h+m, 128c2+n] = out[4r+b(c), n, a(c)*64+m], c = 8h+c2
    o = np.asarray(out_pack).astype(F32).reshape(2, TS, 8, NF)  # [h, m, c2, n]
    for c2 in range(8):
        for h in (0, 1):
            c = 8 * h + c2
            b, a = divmod(c, 4)
            out_full[4 * r + b, :, a * TS:(a + 1) * TS] = o[h, :, c2, :].T


# ---------------------------------------------------------------------------
# Device program
# ---------------------------------------------------------------------------

_prog_cache = {}


def _emit_body(nc, tc, pool, ps_pool, in_d, in2_d, out_d, w2b_d, version=28):
    import concourse.mybir as mybir

    bf = mybir.dt.bfloat16
    f32 = mybir.dt.float32

    # PE warm-up: back-to-back matmuls on a zeroed bf16 tile while the input
    # DMAs stream in; gets the HAM clock gate to 2.4GHz before step 1 starts.
    warm_s = pool.tile([128, 512], bf, tag="warm")
    nc.vector.memset(warm_s[:], 0.0)
    ps_w = ps_pool.tile([128, 512], f32, tag="warm_ps")
    for _ in range(6):
        nc.tensor.matmul(ps_w[:, :], warm_s[:, 0:128], warm_s[:, :],
                         start=True, stop=True)

    # input streams; each chunk that independently gates compute gets its
    # own DMA so its completion semaphore fires as early as possible
    in_s = pool.tile([128, 3072], bf, tag="in")     # [xk_pad | wt p0-15]
    nc.sync.dma_start(in_s[:], in_d.ap())
    wt1_s = pool.tile([128, 2048], bf, tag="wt1")   # wt p16-31
    nc.scalar.dma_start(wt1_s[:], in2_d.ap()[:, 0:2048])
    x0b_s = pool.tile([128, 1152], bf, tag="x0b")   # [x0_pad | bias]
    nc.scalar.dma_start(x0b_s[:], in2_d.ap()[:, 2048:3200])

    xk_pad = in_s[:, 0:1024]
    x0_pad = x0b_s[:, 0:1024]
    bias_s = x0b_s[:, 1024:1152]

    def wt_cols(p):  # rhs tile [128, 128] for pair p
        if p < 16:
            return in_s[:, 1024 + 128 * p:1024 + 128 * (p + 1)]
        return wt1_s[:, 128 * (p - 16):128 * (p - 15)]

    # ---- step 1: W2 = xk . wT, contract y (K = 128 = (q, y)) ----
    w2_s = pool.tile([128, 1024], bf, tag="w2")
    w2r_s = pool.tile([128, 1024], bf, tag="w2r")

    ps1_0 = ps_pool.tile([128, 512], f32, tag="s1_0")
    ps1_1 = ps_pool.tile([128, 512], f32, tag="s1_1")
    ps1s = [ps1_0, ps1_1]
    for u in range(2):
        ps1 = ps1s[u]
        for k in range(4 * u, 4 * u + 4):
            for j in range(4):
                p = 4 * k + j
                nc.tensor.matmul(
                    ps1[32 * j:32 * (j + 1), 128 * (k % 4):128 * (k % 4 + 1)],
                    xk_pad[:, 32 * p:32 * (p + 1)],
                    wt_cols(p),
                    start=True,
                    stop=True,
                    tile_position=(0, 32 * j),
                )
        # split the PSUM cast so only a thin 128-col cast gates the second
        # down DMA (the bulk cast overlaps the last pass's matmuls)
        if u == 0:
            nc.vector.tensor_copy(w2_s[:, 0:512], ps1[:, :])
        else:
            nc.vector.tensor_copy(w2_s[:, 512:896], ps1[:, 0:384])
            nc.vector.tensor_copy(w2_s[:, 896:1024], ps1[:, 384:512])
        # bounce down: L[k, p, n] = w2_s[p, 128k+n]. Per (engine, k) the
        # four consecutive partitions' 256B runs concatenate into 1KB
        # contiguous writes, so this runs near line rate.
        dst = w2b_d.ap()[4 * u:4 * (u + 1)].rearrange("k p n -> p k n")
        src = w2_s[:, 512 * u:512 * (u + 1)].rearrange("p (k n) -> p k n", k=4)
        (nc.sync if u == 0 else nc.scalar).dma_start(dst, src)

    # ---- shuffle readback: with rows p' = 16k+4j+2q+h = 2t+h and the
    # L[k, p, n] bounce layout, the readback is fully contiguous (2KB
    # rows). Split into k-halves (= contiguous row halves): each half is
    # gated only by its own down, so the first half's readback and the
    # first K=64 phase of step 2 overlap the second down/readback ----
    up_src = w2b_d.ap().rearrange("k (ph pl) n -> (k ph) (pl n)", ph=16, pl=8)
    for u in range(2):
        (nc.sync if u == 0 else nc.scalar).dma_start(
            w2r_s[64 * u:64 * (u + 1), :], up_src[64 * u:64 * (u + 1)])

    # ---- step 2: out = x0 . W2, contract t (K = 128 = (2t+h)) in two
    # K=64 accumulation phases so phase A runs while the second half of
    # the bounce is still in flight ----
    out_s = pool.tile([128, 1024], bf, tag="out")
    bias4 = bias_s.unsqueeze(1).broadcast_to([128, 4, 128])
    ps2_0 = ps_pool.tile([128, 512], f32, tag="s2_0")
    ps2_1 = ps_pool.tile([128, 512], f32, tag="s2_1")
    ps2s = [ps2_0, ps2_1]
    for ph in range(2):  # contraction rows 64*ph .. 64*ph+64
        rows = slice(64 * ph, 64 * (ph + 1))
        for u in range(2):
            for c2 in range(4 * u, 4 * u + 4):
                nc.tensor.matmul(
                    ps2s[u][:, 128 * (c2 % 4):128 * (c2 % 4 + 1)],
                    x0_pad[rows, 128 * c2:128 * (c2 + 1)],
                    w2r_s[rows, 128 * c2:128 * (c2 + 1)],
                    start=(ph == 0),
                    stop=(ph == 1),
                )
    for u in range(2):
        nc.vector.tensor_add(
            out_s[:, 512 * u:512 * (u + 1)].rearrange("p (f n) -> p f n", f=4),
            ps2s[u][:, :].rearrange("p (f n) -> p f n", f=4),
            bias4,
        )
        (nc.sync if u == 0 else nc.scalar).dma_start(
            out_d.ap()[:, 512 * u:512 * (u + 1)],
            out_s[:, 512 * u:512 * (u + 1)])


def _build_program(version=28):
    if version in _prog_cache:
        return _prog_cache[version]

    from contextlib import ExitStack

    import concourse.bacc as bacc
    import concourse.mybir as mybir
    import concourse.tile as tile

    bf = mybir.dt.bfloat16
    nc = bacc.Bacc("TRN2", target_bir_lowering=False, debug=False)

    # in  = [xk_pad (1024) | wt pairs 0..15 (2048)]
    # in2 = [wt pairs 16..31 (2048) | x0_pad (1024) | bias (128)]
    in_d = nc.dram_tensor("in_pack", [128, 3072], bf, kind="ExternalInput")
    in2_d = nc.dram_tensor("in2_pack", [128, 3200], bf, kind="ExternalInput")
    out_d = nc.dram_tensor("out_pack", [128, 1024], bf, kind="ExternalOutput")
    # bounce buffer: L[k, p, n] = w2_s[p, 128k+n]
    w2b_d = nc.dram_tensor("w2_bounce", [8, 128, 128], bf)

    with tile.TileContext(nc) as tc, ExitStack() as ctx:
        pool = ctx.enter_context(tc.tile_pool(name="io", bufs=1))
        ps_pool = ctx.enter_context(tc.tile_pool(name="ps", bufs=1, space="PSUM"))
        _emit_body(nc, tc, pool, ps_pool, in_d, in2_d, out_d, w2b_d,
                   version=version)

    nc.compile()
    _prog_cache[version] = nc
    return nc


def pack_core_inputs(x_0, x_k, conv_w, conv_b, version=28):
    """Returns (in_maps list of 8 dicts) for run_bass_kernel_spmd."""
    wt = _pack_wt(np.asarray(conv_w, dtype=F32))
    bias = np.ascontiguousarray(
        np.broadcast_to(np.asarray(conv_b, dtype=F32), (128, 128))
    ).astype(BF16)
    x0 = np.asarray(x_0, dtype=F32)
    xk = np.asarray(x_k, dtype=F32)
    in_maps = []
    for r in range(NCORES):
        in_pack = np.concatenate(
            [_pack_xk(xk[B * r:B * (r + 1)]), wt[:, :2048]], axis=1)
        in2_pack = np.concatenate(
            [wt[:, 2048:], _pack_x0(x0[B * r:B * (r + 1)]), bias], axis=1)
        in_maps.append({
            "in_pack": np.ascontiguousarray(in_pack),
            "in2_pack": np.ascontiguousarray(in2_pack),
        })
    return in_maps


VERSION = 28  # current best variant


def kernel(x_0, x_k, conv_w, conv_b):
    from concourse.bass_utils import run_bass_kernel_spmd

    nc = _build_program(VERSION)
    in_maps = pack_core_inputs(x_0, x_k, conv_w, conv_b, version=VERSION)
    res = run_bass_kernel_spmd(nc, in_maps, core_ids=list(range(NCORES)))
    out = np.empty((BS, NF, F), dtype=F32)
    for r in range(NCORES):
        _unpack_out(res.results[r]["out_pack"], out, r)
    return out


# ---------------------------------------------------------------------------
# numpy model of the packed device program (for testing the packing logic)
# ---------------------------------------------------------------------------

def _numpy_model(x_0, x_k, conv_w, conv_b):
    out = np.empty((BS, NF, F), dtype=F32)
    in_maps = pack_core_inputs(x_0, x_k, conv_w, conv_b)
    for r in range(NCORES):
        m = in_maps[r]
        ip = m["in_pack"].astype(F32)
        ip2 = m["in2_pack"].astype(F32)
        xk_s = ip[:, :1024]
        wt = np.concatenate([ip[:, 1024:3072], ip2[:, :2048]], axis=1)
        x0l = ip2[:, 2048:3072]
        bias = ip2[:, 3072:3200]
        w2 = np.zeros((128, 1024), dtype=F32)
        for k in range(8):
            ps1 = np.zeros((128, 128), dtype=F32)
            for j in range(4):
                p = 4 * k + j
                ps1[32 * j:32 * (j + 1), :] = (
                    xk_s[:, 32 * p:32 * (p + 1)].T @ wt[:, 128 * p:128 * (p + 1)]
                )
            w2[:, 128 * k:128 * (k + 1)] = ps1
        w2 = w2.astype(BF16).astype(F32)  # PSUM->SBUF bf16 cast
        # bounce L[k, p, n] = w2[p, 128k+n]; readback row p' = (k, p>>3),
        # col (p&7, n)
        L = w2.reshape(128, 8, 128).transpose(1, 0, 2)   # [k, p, n]
        w2r = np.ascontiguousarray(L).reshape(128, 1024)
        out_pack = np.empty((128, 1024), dtype=F32)
        for c2 in range(8):
            out_pack[:, 128 * c2:128 * (c2 + 1)] = (
                x0l[:, 128 * c2:128 * (c2 + 1)].T @ w2r[:, 128 * c2:128 * (c2 + 1)]
                + bias
            )
        _unpack_out(out_pack.astype(BF16), out, r)
    return out
